# revision 2
# baseline (speedup 1.0000x reference)
"""Trainium2 Bass kernel computing out = x * exp(diagonal).

x: (8192, 4096) float32, diagonal: (4096,) float32.
Data-parallel across 8 NeuronCores: each core handles 1024 rows of x;
the 4096-float diagonal is replicated to every core.

Per-core program (pure streaming, memory-bound). TRN2 compute/DMA
instructions only carry ONE sync-wait command, and Tile has 8 HWDGE
completion-sem lanes, so the program is shaped to need at most one wait
per instruction and at most 8 HWDGE DMAs (no lane reuse):

  1. exp(diagonal) broadcast tile [128, 4096] built via a stride-0
     SWDGE DMA from DRAM (separate sem lanes) + ACT Exp.
  2. A 1-element DVE copy observes the Exp so later muls don't need a
     second wait on it.
  3. x streams through 4 fresh [128, 8192] SBUF tiles (half the 16 MiB
     shard resident at once, no slot reuse => no WAR waits):
     HWDGE load on SP -> in-place DVE multiply (the exp-vector operand
     is free-dim-broadcast 2x) -> HWDGE store on ACT.
"""

import numpy as np

BATCH, FEAT = 8192, 4096
N_CORES = 8
ROWS = BATCH // N_CORES   # 1024 rows per core
P = 128                   # SBUF partitions
FOLD = 2                  # row-blocks folded into one tile's free dim
N_TILES = ROWS // (P * FOLD)  # 4 tiles of [128, FOLD*4096] per core

_CACHE = {}


def build_nc(rows=ROWS, feat=FEAT, fold=FOLD):
    import concourse.bacc as bacc
    import concourse.mybir as mybir
    from concourse import tile

    # Bacc (not plain Bass): its compile() pass splits multi-sem waits into
    # EventSemaphore chains -- TRN2 instructions carry at most one wait.
    nc = bacc.Bacc("TRN2", target_bir_lowering=False, debug=False)
    x = nc.dram_tensor("x", (rows, feat), mybir.dt.float32, kind="ExternalInput").ap()
    d = nc.dram_tensor("d", (feat,), mybir.dt.float32, kind="ExternalInput").ap()
    out = nc.dram_tensor(
        "out", (rows, feat), mybir.dt.float32, kind="ExternalOutput"
    ).ap()

    n_tiles = rows // (P * fold)
    x_t = x.rearrange("(s n p) m -> s p n m", p=P, n=fold)
    o_t = out.rearrange("(s n p) m -> s p n m", p=P, n=fold)
    d_row = d.rearrange("(r c) -> r c", r=1)

    with tile.TileContext(nc) as tc:
        with (
            tc.tile_pool(name="const", bufs=1) as cpool,
            tc.tile_pool(name="io", bufs=n_tiles) as iopool,
        ):
            expd = cpool.tile([P, feat], mybir.dt.float32)
            nc.gpsimd.dma_start(expd[:], d_row.to_broadcast((P, feat)))
            nc.scalar.activation(expd[:], expd[:], mybir.ActivationFunctionType.Exp)
            # DVE observer: absorbs the wait on the Exp so the muls below
            # carry exactly one wait (their own load DMA).
            scratch = cpool.tile([1, 1], mybir.dt.float32)
            nc.vector.tensor_copy(scratch[:], expd[0:1, 0:1])
            # exp vector broadcast FOLD x along the free dim (stride 0)
            expd_b = expd[:].rearrange("p (o m) -> p o m", o=1).to_broadcast(
                (P, fold, feat)
            )

            tiles = []
            for i in range(n_tiles):
                t = iopool.tile([P, fold * feat], mybir.dt.float32)
                t3 = t.rearrange("p (n m) -> p n m", n=fold)
                nc.sync.dma_start(t3, x_t[i])
                tiles.append(t3)
            for i, t3 in enumerate(tiles):
                nc.vector.tensor_mul(t3, t3, expd_b)
                nc.scalar.dma_start(o_t[i], t3)
    nc.finalize()
    return nc


def kernel(x, diagonal):
    from concourse.bass_utils import run_bass_kernel_spmd

    if "nc" not in _CACHE:
        _CACHE["nc"] = build_nc()
    nc = _CACHE["nc"]

    x = np.ascontiguousarray(x, dtype=np.float32)
    d = np.ascontiguousarray(diagonal, dtype=np.float32)
    in_maps = [{"x": x[c * ROWS : (c + 1) * ROWS], "d": d} for c in range(N_CORES)]
    res = run_bass_kernel_spmd(nc, in_maps, core_ids=list(range(N_CORES)))
    _CACHE["last_res"] = res
    return np.concatenate([r["out"] for r in res.results], axis=0)



# revision 3
# speedup vs baseline: 1.0330x; 1.0330x over previous
"""Trainium2 Bass kernel computing out = x * exp(diagonal).

x: (8192, 4096) float32, diagonal: (4096,) float32.
Data-parallel across 8 NeuronCores: each core handles 1024 rows of x;
the 4096-float diagonal is replicated to every core.

Per-core program (pure streaming, SDMA-fabric-bound: the 16 SDMA
engines aggregate ~432 GB/s, so 32 MiB of x in+out traffic floors at
~78 us; everything else must hide under that):

  1. diagonal loads as a [1, 4096] tile via one tiny HWDGE DMA on the
     ACT queue (keeps the SP queue free for x loads), ACT computes
     exp in place.
  2. Partition-broadcast WITHOUT any DMA: TensorE outer-product
     ones[1,128]^T @ expd[1,4096] -> PSUM [128, 4096] (8 matmuls, one
     per 512-float PSUM bank). Zero HBM/fabric bytes, runs under the
     first x load. A 1-element DVE copy observes the last matmul so
     the muls below carry exactly one wait (their own load DMA).
  3. x streams through 8 fresh [128, 4096] SBUF tiles (no slot reuse
     => no WAR waits): HWDGE load on SP -> in-place DVE multiply with
     operand b read from PSUM (only 2 SBUF accesses/elem -> full
     245 G elem/s) -> HWDGE store on ACT.
"""

import numpy as np

BATCH, FEAT = 8192, 4096
N_CORES = 8
ROWS = BATCH // N_CORES   # 1024 rows per core
P = 128                   # SBUF partitions
N_TILES = ROWS // P       # 8 tiles of [128, 4096] per core
PSUM_BANK = 512           # fp32 elems per PSUM bank (2 KiB)

_CACHE = {}


def build_nc(rows=ROWS, feat=FEAT):
    import concourse.bacc as bacc
    import concourse.mybir as mybir
    from concourse import tile

    # Bacc (not plain Bass): its compile() pass splits multi-sem waits into
    # EventSemaphore chains -- TRN2 instructions carry at most one wait.
    nc = bacc.Bacc("TRN2", target_bir_lowering=False, debug=False)
    x = nc.dram_tensor("x", (rows, feat), mybir.dt.float32, kind="ExternalInput").ap()
    d = nc.dram_tensor("d", (feat,), mybir.dt.float32, kind="ExternalInput").ap()
    out = nc.dram_tensor(
        "out", (rows, feat), mybir.dt.float32, kind="ExternalOutput"
    ).ap()

    n_tiles = rows // P
    x_t = x.rearrange("(s p) m -> s p m", p=P)
    o_t = out.rearrange("(s p) m -> s p m", p=P)
    d_row = d.rearrange("(r c) -> r c", r=1)

    with tile.TileContext(nc) as tc:
        with (
            tc.tile_pool(name="const", bufs=1) as cpool,
            tc.tile_pool(name="psum", bufs=1, space="PSUM") as ppool,
            tc.tile_pool(name="io", bufs=n_tiles) as iopool,
        ):
            d1 = cpool.tile([1, feat], mybir.dt.float32)
            ones = cpool.tile([1, P], mybir.dt.float32)
            expd = ppool.tile([P, feat], mybir.dt.float32)

            # diagonal -> [1, feat] on the ACT HWDGE queue; exp in place.
            nc.scalar.dma_start(d1[:], d_row)
            nc.scalar.activation(d1[:], d1[:], mybir.ActivationFunctionType.Exp)
            nc.vector.memset(ones[:], 1.0)
            # Broadcast across partitions: ones^T @ expd, one matmul per
            # PSUM bank (512 fp32).
            for b in range(feat // PSUM_BANK):
                sl = slice(b * PSUM_BANK, (b + 1) * PSUM_BANK)
                nc.tensor.matmul(
                    expd[:, sl], ones[:], d1[:, sl], start=True, stop=True
                )
            # DVE observer: absorbs the wait on the matmuls so the muls
            # below carry exactly one wait (their own load DMA).
            scratch = cpool.tile([1, 1], mybir.dt.float32)
            nc.vector.tensor_copy(scratch[:], expd[0:1, 0:1])

            tiles = []
            for i in range(n_tiles):
                t = iopool.tile([P, feat], mybir.dt.float32)
                nc.sync.dma_start(t[:], x_t[i])
                tiles.append(t)
            for i, t in enumerate(tiles):
                nc.vector.tensor_mul(t[:], t[:], expd[:])
                nc.scalar.dma_start(o_t[i], t[:])
    nc.finalize()
    return nc


def kernel(x, diagonal):
    from concourse.bass_utils import run_bass_kernel_spmd

    if "nc" not in _CACHE:
        _CACHE["nc"] = build_nc()
    nc = _CACHE["nc"]

    x = np.ascontiguousarray(x, dtype=np.float32)
    d = np.ascontiguousarray(diagonal, dtype=np.float32)
    in_maps = [{"x": x[c * ROWS : (c + 1) * ROWS], "d": d} for c in range(N_CORES)]
    res = run_bass_kernel_spmd(nc, in_maps, core_ids=list(range(N_CORES)))
    _CACHE["last_res"] = res
    return np.concatenate([r["out"] for r in res.results], axis=0)
